# revision 1
# baseline (speedup 1.0000x reference)
"""Trainium2 Bass kernel for 2-layer GraphSAGE (mean aggregation), v2.

8-core SPMD, nodes load-balanced across 824 windows of 128 (greedy by degree
so every window needs exactly K=12 edge-tiles), 103 windows/core.

- Host: assigns nodes to (core, window, slot), slots each window's in-edges
  (sorted by source table row for HBM locality) into K 128-edge tiles,
  pre-gathers layer-1 messages x[src] into [P, T*64] bf16.
- Device layer 1: stream pre-gathered messages; segment-sum via one-hot
  indicator matmuls (M[e,r] = (dst_e==r)*w_e, built on DVE/GpSimd from an
  iota tile), PSUM-accumulated per 512-node super, W1l/W1r matmuls + bias +
  ReLU in [feat, node] orientation.
- h1 transposed to row layout via HWDGE xbar DMA-transposes (pipelined
  under layer-1 compute), AllGather -> full [105472, 64] bf16 table per core
  (Shared address space).
- Device layer 2: one dma_gather per window (512B quads, int16 idx=row//4,
  mlp ext-isa library, single_packet=False), 4-way in-quad row selection via
  host-masked dstloc variants folded into the one-hot aggregation matmuls,
  W2l/W2r matmuls + bias; output written feature-major [64, WROWS] fp32;
  host transposes/un-permutes.
"""
import sys

sys.path.insert(0, '/opt/trn_rl_repo')
import heapq

import numpy as np
import ml_dtypes

BF16 = ml_dtypes.bfloat16
N = 100000
D = 64
NCORES = 8
P = 128
NW = 103                    # windows per core (load ~1517 <= 12*128 -> K=12)
WROWS = NW * P              # 13184 padded local rows
NWIN = NCORES * NW          # 784 global windows
TBL_ROWS = NCORES * WROWS   # 100352 rows in the gathered h1 table

CHUNK_W = 12                # windows per streamed/gathered chunk
SUPER_W = 2                 # windows per PSUM super (must divide CHUNK_W)
L1_POOL_EVERY = 2           # in layer 1, every n-th M-build goes to GpSimd
L2_POOL_EVERY = 0           # in layer 2 GpSimd is busy with gathers


def _balance_nodes(deg):
    """Assign nodes to NWIN windows of <=128 nodes, balancing degree sums.

    Returns (win, slot) int32 arrays of shape [N]."""
    order = np.argsort(-deg, kind='stable')
    win = np.empty(N, dtype=np.int32)
    slot = np.empty(N, dtype=np.int32)
    counts = np.zeros(NWIN, dtype=np.int32)
    heap = [(0, w) for w in range(NWIN)]
    heapq.heapify(heap)
    for n in order:
        while True:
            load, w = heapq.heappop(heap)
            if counts[w] < P:
                break
        win[n] = w
        slot[n] = counts[w]
        counts[w] += 1
        load += int(deg[n])
        if counts[w] < P:
            heapq.heappush(heap, (load, w))
    return win, slot


def _prep(x, src, dst, inv, win, slot):
    """Slot all edges into per-core tile arrays. Returns per-core dicts."""
    # destination placement
    dwin = win[dst]                    # global window of each edge's dst
    dslot = slot[dst]
    # source table row: core*WROWS + slot*NW + local_win (partition-major)
    srow = (win[src] // NW) * WROWS + slot[src] * NW + (win[src] % NW)

    K_per_win = np.bincount(dwin, minlength=NWIN)
    loc_per_win = np.bincount(
        dwin[(win[src] // NW) == (dwin // NW)], minlength=NWIN)
    rest_per_win = K_per_win - np.minimum(loc_per_win, P)
    K = max(int(np.max(1 + (rest_per_win + P - 1) // P)),
            int(np.max((K_per_win + P - 1) // P)), 1)
    T = NW * K

    outs = []
    # order edges by (window, local-source-first, src table row): tile 0 of
    # each window holds only local-source edges (their h1 rows exist in the
    # core's own h1loc table before the AllGather completes), so its gather
    # can overlap the collective. Short windows pad tile 0 (dstloc=-1).
    is_remote = (win[src] // NW) != (dwin // NW)
    order = np.lexsort((srow, is_remote, dwin))
    dwin_s = dwin[order]
    dslot_s = dslot[order]
    srow_s = srow[order]
    esrc_s = src[order]
    rem_s = is_remote[order]
    w_s = inv[dst[order]]
    starts = np.searchsorted(dwin_s, np.arange(NWIN + 1))

    for c in range(NCORES):
        slots_srow = np.zeros(T * P, dtype=np.int32)
        slots_dstloc = np.full(T * P, -1.0, dtype=np.float32)
        slots_w = np.zeros(T * P, dtype=np.float32)
        slots_esrc = np.zeros(T * P, dtype=np.int64)
        for wl in range(NW):
            g = c * NW + wl
            s0, s1 = starts[g], starts[g + 1]
            base = wl * K * P
            nloc = int(np.searchsorted(rem_s[s0:s1], 1))
            # tile 0: up to 128 local-source edges, rest padding
            n0 = min(nloc, P)
            sl0 = slice(s0, s0 + n0)
            slots_srow[base:base + n0] = srow_s[sl0]
            slots_dstloc[base:base + n0] = dslot_s[sl0]
            slots_w[base:base + n0] = w_s[sl0]
            slots_esrc[base:base + n0] = esrc_s[sl0]
            # tiles 1..K-1: remaining edges (local overflow + remote)
            rest = s1 - s0 - n0
            sl1 = slice(s0 + n0, s1)
            slots_srow[base + P:base + P + rest] = srow_s[sl1]
            slots_dstloc[base + P:base + P + rest] = dslot_s[sl1]
            slots_w[base + P:base + P + rest] = w_s[sl1]
            slots_esrc[base + P:base + P + rest] = esrc_s[sl1]

        def to_pt(a, dt):
            return np.ascontiguousarray(a.reshape(T, P).T.astype(dt))

        msgs = x[slots_esrc].astype(BF16)          # [T*P, 64]
        msgs_pt = np.ascontiguousarray(
            msgs.reshape(T, P, D).transpose(1, 0, 2).reshape(P, T * D))
        dstloc_pt = to_pt(slots_dstloc, np.float32)
        # quad-gather inputs: idx = srow//4 (int16, wrapped [i%16, i//16]
        # per window, tiled x8 over partitions); 4 masked dstloc variants
        # select the row within each 512B quad.
        j_pt = to_pt(slots_srow % 4, np.int32)
        dstlocj = np.concatenate(
            [np.where(j_pt == j, dstloc_pt, -1.0) for j in range(4)],
            axis=1).astype(np.float32)
        v = (slots_srow // 4).astype(np.int16).reshape(NW, K, P)
        qidx = np.concatenate(
            [np.tile(np.transpose(v[w].reshape(K, 8, 16), (2, 0, 1))
                     .reshape(16, K * 8), (8, 1)) for w in range(NW)], axis=1)
        outs.append({
            "msgs": msgs_pt,
            "dstloc": dstloc_pt,
            "dstlocj": np.ascontiguousarray(dstlocj),
            "qidx": np.ascontiguousarray(qidx),
            "wts": to_pt(slots_w, np.float32),
        })
    return outs, K


def _build_program(K):
    import concourse.bass as bass
    import concourse.tile as tile
    from concourse import bacc, mybir

    T = NW * K
    nc = bacc.Bacc("TRN2", target_bir_lowering=False, debug=False,
                   num_devices=NCORES)
    dt = mybir.dt

    msgs_d = nc.dram_tensor("msgs", [P, T * D], dt.bfloat16, kind="ExternalInput")
    dstloc_d = nc.dram_tensor("dstloc", [P, T], dt.float32, kind="ExternalInput")
    wts_d = nc.dram_tensor("wts", [P, T], dt.float32, kind="ExternalInput")
    dstlocj_d = nc.dram_tensor("dstlocj", [P, 4 * T], dt.float32,
                               kind="ExternalInput")
    qidx_d = nc.dram_tensor("qidx", [P, NW * K * 8], dt.int16,
                            kind="ExternalInput")
    xT_d = nc.dram_tensor("xT", [D, WROWS], dt.bfloat16, kind="ExternalInput")
    iota_d = nc.dram_tensor("iota", [P, P], dt.bfloat16, kind="ExternalInput")
    w1l_d = nc.dram_tensor("w1lT", [D, D], dt.bfloat16, kind="ExternalInput")
    w1r_d = nc.dram_tensor("w1rT", [D, D], dt.bfloat16, kind="ExternalInput")
    w2l_d = nc.dram_tensor("w2lT", [D, D], dt.bfloat16, kind="ExternalInput")
    w2r_d = nc.dram_tensor("w2rT", [D, D], dt.bfloat16, kind="ExternalInput")
    b1_d = nc.dram_tensor("b1c", [D, 1], dt.float32, kind="ExternalInput")
    b2_d = nc.dram_tensor("b2c", [D, 1], dt.float32, kind="ExternalInput")
    # output is FEATURE-MAJOR [64, WROWS]; host transposes
    out_d = nc.dram_tensor("out", [D, WROWS], dt.float32, kind="ExternalOutput")

    # chunks of CHUNK_W windows; supers of SUPER_W windows inside each chunk
    chunks = []
    w0 = 0
    while w0 < NW:
        cw = min(CHUNK_W, NW - w0)
        chunks.append((w0, cw))
        w0 += cw

    with tile.TileContext(nc) as tc:
        with (
            tc.tile_pool(name="const", bufs=1) as cpool,
            tc.tile_pool(name="chunks", bufs=2) as chpool,
            tc.tile_pool(name="gq", bufs=4) as gqpool,
            tc.tile_pool(name="mtiles", bufs=16) as mpool,
            tc.tile_pool(name="small", bufs=3) as spool,
            tc.tile_pool(name="ostage", bufs=3) as opool,
            tc.tile_pool(name="psA", bufs=2, space="PSUM") as psA,
            tc.tile_pool(name="psB", bufs=2, space="PSUM") as psB,
            tc.tile_pool(name="psT", bufs=2, space="PSUM") as psT,
            tc.tile_pool(name="dram", bufs=1, space="DRAM") as dpool,
        ):
            dstloc_sb = cpool.tile([P, T], dt.float32, tag="dstloc")
            wts_sb = cpool.tile([P, T], dt.float32, tag="wts")
            dstlocj_sb = cpool.tile([P, 4 * T], dt.float32, tag="dstlocj")
            qidx_sb = cpool.tile([P, NW * K * 8], dt.int16, tag="qidx")
            xT_sb = cpool.tile([D, WROWS], dt.bfloat16, tag="xT")
            iota_sb = cpool.tile([P, P], dt.bfloat16, tag="iota")
            w1l_sb = cpool.tile([D, D], dt.bfloat16, tag="w1l")
            w1r_sb = cpool.tile([D, D], dt.bfloat16, tag="w1r")
            w2l_sb = cpool.tile([D, D], dt.bfloat16, tag="w2l")
            w2r_sb = cpool.tile([D, D], dt.bfloat16, tag="w2r")
            b1_sb = cpool.tile([D, 1], dt.float32, tag="b1")
            b2_sb = cpool.tile([D, 1], dt.float32, tag="b2")
            h1T_sb = cpool.tile([D, WROWS], dt.bfloat16, tag="h1T")
            h1rows_sb = cpool.tile([P, NW * D], dt.bfloat16, tag="h1rows")

            # only iota/dstloc/wts gate the first M-builds; defer the rest
            # until after chunk 0's stream DMA so compute starts sooner
            for t_sb, t_d in [(iota_sb, iota_d), (dstloc_sb, dstloc_d),
                              (wts_sb, wts_d)]:
                nc.sync.dma_start(out=t_sb[:], in_=t_d.ap())
            deferred_consts = [(xT_sb, xT_d), (w1l_sb, w1l_d),
                               (w1r_sb, w1r_d), (b1_sb, b1_d),
                               (w2l_sb, w2l_d), (w2r_sb, w2r_d),
                               (b2_sb, b2_d), (dstlocj_sb, dstlocj_d),
                               (qidx_sb, qidx_d)]

            h1loc_dram = dpool.tile([WROWS, D], dt.bfloat16, tag="h1loc")
            h1full_dram = dpool.tile([TBL_ROWS, D], dt.bfloat16, tag="h1full",
                                     addr_space="Shared")

            def build_M(t, engine):
                mt = mpool.tile([P, P], dt.bfloat16, tag="M")
                engine.tensor_scalar(
                    out=mt[:], in0=iota_sb[:],
                    scalar1=dstloc_sb[:, t:t + 1],
                    scalar2=wts_sb[:, t:t + 1],
                    op0=mybir.AluOpType.is_equal,
                    op1=mybir.AluOpType.mult)
                return mt

            # ---------------- layer 1 ----------------
            mb_count = 0
            for w0, cw in chunks:
                ch = chpool.tile([P, CHUNK_W * K * D], dt.bfloat16, tag="ch")
                nc.sync.dma_start(
                    out=ch[:, :cw * K * D],
                    in_=msgs_d.ap()[:, w0 * K * D:(w0 + cw) * K * D])
                if w0 == 0:
                    for t_sb, t_d in deferred_consts:
                        nc.sync.dma_start(out=t_sb[:], in_=t_d.ap())
                s0 = 0
                while s0 < cw:
                    sw = min(SUPER_W, cw - s0)
                    agg_ps = psA.tile([D, SUPER_W * P], dt.float32, tag="agg")
                    for s in range(sw):
                        wi = w0 + s0 + s
                        for k in range(K):
                            t = wi * K + k
                            if L1_POOL_EVERY and mb_count % L1_POOL_EVERY == 0:
                                mt = build_M(t, nc.gpsimd)
                            else:
                                mt = build_M(t, nc.vector)
                            mb_count += 1
                            woff = s0 + s
                            nc.tensor.matmul(
                                out=agg_ps[:, s * P:(s + 1) * P],
                                lhsT=ch[:, (woff * K + k) * D:(woff * K + k + 1) * D],
                                rhs=mt[:], start=(k == 0), stop=(k == K - 1))
                    agg_sb = spool.tile([D, SUPER_W * P], dt.bfloat16, tag="aggsb")
                    nc.scalar.copy(out=agg_sb[:, :sw * P], in_=agg_ps[:, :sw * P])
                    h_ps = psB.tile([D, SUPER_W * P], dt.float32, tag="hps")
                    wabs = w0 + s0
                    nc.tensor.matmul(out=h_ps[:, :sw * P], lhsT=w1l_sb[:],
                                     rhs=agg_sb[:, :sw * P], start=True, stop=False)
                    nc.tensor.matmul(out=h_ps[:, :sw * P], lhsT=w1r_sb[:],
                                     rhs=xT_sb[:, wabs * P:(wabs + sw) * P],
                                     start=False, stop=True)
                    nc.scalar.activation(
                        out=h1T_sb[:, wabs * P:(wabs + sw) * P],
                        in_=h_ps[:, :sw * P],
                        func=mybir.ActivationFunctionType.Relu, bias=b1_sb[:])
                    s0 += sw

            # transpose h1T -> rows (HWDGE xbar, overlaps L1 compute), stage,
            # one DMA to DRAM, AllGather.
            # Table row layout: local row p*NW + t (partition-major).
            for wi in range(NW):
                nc.sync.dma_start_transpose(
                    out=h1rows_sb[:, wi * D:(wi + 1) * D],
                    in_=h1T_sb[:, wi * P:(wi + 1) * P])
            nc.sync.dma_start(
                out=h1loc_dram[:].rearrange("(p t) f -> p (t f)", p=P),
                in_=h1rows_sb[:])

            # ---------------- layer 2 ----------------
            # Batched quad gathers (validated in gtest12.py): one dma_gather
            # per window fetches all K*128 rows as 512B quads (int16 idx =
            # row//4; single_packet=False above 64 descriptors; needs the
            # mlp ext-isa library). The 4-way row selection within each quad
            # is folded into 4 host-masked dstloc variants: 4 one-hot builds
            # + 4 PSUM-accumulated matmuls per tile, with lhsT picking the
            # 64-col sub-row of the quad.
            nc.gpsimd.collective_compute(
                "AllGather", mybir.AluOpType.bypass,
                replica_groups=[list(range(NCORES))],
                ins=[h1loc_dram[:]], outs=[h1full_dram[:]])
            from concourse import library_config
            nc.gpsimd.load_library(library_config.mlp)
            QELEM = 4 * D
            fullq = h1full_dram[:].rearrange("(q r) f -> q (r f)", r=4)

            def build_Mj(t, j, engine):
                mt = mpool.tile([P, P], dt.bfloat16, tag="M")
                engine.tensor_scalar(
                    out=mt[:], in0=iota_sb[:],
                    scalar1=dstlocj_sb[:, j * T + t:j * T + t + 1],
                    scalar2=wts_sb[:, t:t + 1],
                    op0=mybir.AluOpType.is_equal,
                    op1=mybir.AluOpType.mult)
                return mt
            mb_count = 0
            for w0, cw in chunks:
                s0 = 0
                while s0 < cw:
                    sw = min(SUPER_W, cw - s0)
                    agg_ps = psA.tile([D, SUPER_W * P], dt.float32, tag="agg")
                    for s in range(sw):
                        wi = w0 + s0 + s
                        gq = gqpool.tile([P, K * QELEM], dt.bfloat16, tag="gq")
                        nc.gpsimd.dma_gather(
                            gq[:].rearrange("p (c e) -> p c e", e=QELEM),
                            fullq, qidx_sb[:, wi * K * 8:(wi + 1) * K * 8],
                            K * P, K * P, QELEM, single_packet=False)
                        for k in range(K):
                            t = wi * K + k
                            for j in range(4):
                                if mb_count % 3 == 0:
                                    mt = build_Mj(t, j, nc.gpsimd)
                                else:
                                    mt = build_Mj(t, j, nc.vector)
                                mb_count += 1
                                nc.tensor.matmul(
                                    out=agg_ps[:, s * P:(s + 1) * P],
                                    lhsT=gq[:, k * QELEM + j * D:
                                            k * QELEM + (j + 1) * D],
                                    rhs=mt[:],
                                    start=(k == 0 and j == 0),
                                    stop=(k == K - 1 and j == 3))
                    agg_sb = spool.tile([D, SUPER_W * P], dt.bfloat16, tag="aggsb")
                    nc.scalar.copy(out=agg_sb[:, :sw * P], in_=agg_ps[:, :sw * P])
                    h_ps = psB.tile([D, SUPER_W * P], dt.float32, tag="hps")
                    wabs = w0 + s0
                    nc.tensor.matmul(out=h_ps[:, :sw * P], lhsT=w2l_sb[:],
                                     rhs=agg_sb[:, :sw * P], start=True, stop=False)
                    nc.tensor.matmul(out=h_ps[:, :sw * P], lhsT=w2r_sb[:],
                                     rhs=h1T_sb[:, wabs * P:(wabs + sw) * P],
                                     start=False, stop=True)
                    ot = opool.tile([D, SUPER_W * P], dt.float32, tag="ostage")
                    nc.scalar.add(out=ot[:, :sw * P], in_=h_ps[:, :sw * P],
                                  add=b2_sb[:])
                    nc.sync.dma_start(
                        out=out_d.ap()[:, wabs * P:(wabs + sw) * P],
                        in_=ot[:, :sw * P])
                    s0 += sw

    nc.compile()
    return nc


def _host_prep(x, edge_index):
    x = np.asarray(x, dtype=np.float32)
    edge_index = np.asarray(edge_index)
    src = edge_index[0].astype(np.int64)
    dst = edge_index[1].astype(np.int64)
    cnt = np.bincount(dst, minlength=N).astype(np.float32)
    inv = (1.0 / np.maximum(cnt, 1.0)).astype(np.float32)
    win, slot = _balance_nodes(cnt.astype(np.int64))
    per_core, K = _prep(x, src, dst, inv, win, slot)

    # xT per core: col wi*128 + p = x[node at (win c*NW+wi, slot p)]
    colmap = np.full((NCORES, NW * P), -1, dtype=np.int64)  # -> node id
    colmap[win // NW, (win % NW) * P + slot] = np.arange(N)
    for c in range(NCORES):
        cm = colmap[c]
        xT = np.zeros((D, WROWS), dtype=BF16)
        used = cm >= 0
        xT[:, used] = x[cm[used]].T.astype(BF16)
        per_core[c]["xT"] = xT
    return per_core, K, win, slot


def kernel(x, edge_index, W1l, W1r, b1, W2l, W2r, b2):
    from concourse import bass_utils

    per_core, K, win, slot = _host_prep(x, edge_index)

    iota = np.tile(np.arange(P, dtype=np.float32), (P, 1)).astype(BF16)
    id64 = np.eye(D, dtype=np.float32)
    common = {
        "iota": iota, "id64": id64.astype(BF16),
        "w1lT": np.asarray(W1l, np.float32).T.astype(BF16).copy(),
        "w1rT": np.asarray(W1r, np.float32).T.astype(BF16).copy(),
        "w2lT": np.asarray(W2l, np.float32).T.astype(BF16).copy(),
        "w2rT": np.asarray(W2r, np.float32).T.astype(BF16).copy(),
        "b1c": np.asarray(b1, np.float32).reshape(D, 1).copy(),
        "b2c": np.asarray(b2, np.float32).reshape(D, 1).copy(),
    }
    in_maps = [{**common, **pc} for pc in per_core]

    nc = _build_program(K)
    res = bass_utils.run_bass_kernel_spmd(nc, in_maps, list(range(NCORES)))

    # out[c] is [64, WROWS] feature-major, col wi*128+p
    out = np.empty((N, D), dtype=np.float32)
    cols = (win % NW) * P + slot          # column of each node on its core
    cores = win // NW
    for c in range(NCORES):
        m = cores == c
        out[m] = res.results[c]["out"][:, cols[m]].T
    return out



# revision 6
# speedup vs baseline: 1.9045x; 1.9045x over previous
"""Trainium2 Bass kernel for 2-layer GraphSAGE (mean aggregation), v3.

8-core SPMD, 64-node windows (1648 global, 206/core), layer-1 pull with
host-pregathered messages, layer-2 push over local sources with a
ReduceScatter:

- L1: host slots each window's in-edges into K1 128-edge tiles and
  pre-gathers x[src] bf16; device builds [128,64] one-hot M tiles on DVE
  (is_equal x weight), PSUM-accumulates feature-major agg per 4-window
  super, W1 matmuls + bias + ReLU -> h1T [64, 13184] bf16.
- h1 rows: HWDGE dma-transposes per window pair -> h1rows [128, 103*64];
  one strided DMA writes a 256B-padded local row table [13184, 128] bf16.
- L2: edges grouped by SOURCE core; per global dst window, gather local
  h1 rows by int16 row index (batched dma_gather, 256B elems), build
  [128,64] one-hot M2 (dst slot x 1/cnt) and matmul with M2 as lhsT ->
  ROW-major partials [64r, 64f]; two windows pack one PSUM [128,*] tile;
  staged bf16 partials DMA to a [105472, 64] table; one ReduceScatter
  leaves each core its own rows' agg2.
- Final: load agg2 rows, PE-transpose per pair to feature-major, W2
  matmuls + bias -> out [64, 13184] fp32; host un-permutes.
"""
import sys

sys.path.insert(0, '/opt/trn_rl_repo')
import heapq

import numpy as np
import ml_dtypes

BF16 = ml_dtypes.bfloat16
N = 100000
D = 64
NCORES = 8
P = 128
WIN = 64                    # nodes per window
WPC = 206                   # windows per core
PAIRS = WPC // 2            # 103 window pairs per core
WROWS = WPC * WIN           # 13184 local rows
NWIN = NCORES * WPC         # 1648 global windows
TBL_ROWS = NCORES * WROWS   # 105472 rows in the partial table

CHUNK1_W = 12               # L1 windows per streamed msgs chunk
SUPER1_W = 4                # L1 windows per PSUM super (<= 4; 4*64=256 cols)
L2_TILE_CAP = 104           # max edge-tiles per L2 gather chunk
L2_GROUP_PAIRS = 4          # window pairs per L2 PSUM tile ([128, 4*64])


def _balance_nodes(deg):
    """Assign nodes to NWIN windows of <=WIN nodes, balancing degree sums."""
    order = np.argsort(-deg, kind='stable')
    win = np.empty(N, dtype=np.int32)
    slot = np.empty(N, dtype=np.int32)
    counts = np.zeros(NWIN, dtype=np.int32)
    heap = [(0, w) for w in range(NWIN)]
    heapq.heapify(heap)
    for n in order:
        while True:
            load, w = heapq.heappop(heap)
            if counts[w] < WIN:
                break
        win[n] = w
        slot[n] = counts[w]
        counts[w] += 1
        load += int(deg[n])
        if counts[w] < WIN:
            heapq.heappush(heap, (load, w))
    return win, slot


def _wrap_idx(flat):
    """int16 gather index layout: per 128-idx tile, idx j -> partition j%16
    (replicated x8 across 128 partitions), column j//16."""
    nt = flat.shape[0] // P
    v = flat.reshape(nt, 8, 16).transpose(2, 0, 1).reshape(16, nt * 8)
    return np.tile(v, (8, 1)).astype(np.int16)


def _host_prep(x, edge_index):
    x = np.asarray(x, dtype=np.float32)
    edge_index = np.asarray(edge_index)
    src = edge_index[0].astype(np.int64)
    dst = edge_index[1].astype(np.int64)
    cnt = np.bincount(dst, minlength=N).astype(np.float32)
    inv = (1.0 / np.maximum(cnt, 1.0)).astype(np.float32)
    win, slot = _balance_nodes(cnt.astype(np.int64))
    srow = (win % WPC) * WIN + slot            # local table row of each node
    score = win // WPC                          # owning core of each node

    # ---- L1 (pull): per-core edge tiles grouped by dst window ----
    dwin = win[dst]
    wcnt = np.bincount(dwin, minlength=NWIN)
    K1 = max(1, int(np.max((wcnt + P - 1) // P)))
    T1 = WPC * K1

    order1 = np.lexsort((srow[src], dwin))
    dwin_s = dwin[order1]
    dslot_s = slot[dst][order1].astype(np.float32)
    w_s = inv[dst][order1]
    esrc_s = src[order1]
    starts1 = np.searchsorted(dwin_s, np.arange(NWIN + 1))

    # ---- L2 (push): per-core edges grouped by global dst window ----
    ecore = score[src]                          # core owning the edge source
    key2 = ecore.astype(np.int64) * NWIN + dwin
    order2 = np.lexsort((srow[src], key2))
    k2_s = key2[order2]
    q_s = srow[src][order2].astype(np.int16)    # local src row (gather idx)
    dslot2_s = slot[dst][order2].astype(np.float32)
    w2_s = inv[dst][order2]
    starts2 = np.searchsorted(k2_s, np.arange(NCORES * NWIN + 1))
    cnt2 = (starts2[1:] - starts2[:-1]).reshape(NCORES, NWIN)
    K2 = np.maximum(1, (cnt2.max(axis=0) + P - 1) // P).astype(np.int32)  # [NWIN]
    tile0 = np.zeros(NWIN + 1, dtype=np.int64)
    tile0[1:] = np.cumsum(K2)
    T2 = int(tile0[-1])

    per_core = []
    for c in range(NCORES):
        # L1 slots
        s_dstloc = np.full(T1 * P, -1.0, dtype=np.float32)
        s_w = np.zeros(T1 * P, dtype=np.float32)
        s_esrc = np.zeros(T1 * P, dtype=np.int64)
        for wl in range(WPC):
            g = c * WPC + wl
            s0, s1 = starts1[g], starts1[g + 1]
            n = s1 - s0
            base = wl * K1 * P
            s_dstloc[base:base + n] = dslot_s[s0:s1]
            s_w[base:base + n] = w_s[s0:s1]
            s_esrc[base:base + n] = esrc_s[s0:s1]

        def to_pt(a, dt, T):
            return np.ascontiguousarray(a.reshape(T, P).T.astype(dt))

        msgs = x[s_esrc].astype(BF16)
        msgs_pt = np.ascontiguousarray(
            msgs.reshape(T1, P, D).transpose(1, 0, 2).reshape(P, T1 * D))

        # L2 slots
        q2 = np.zeros(T2 * P, dtype=np.int16)
        s_dstloc2 = np.full(T2 * P, -1.0, dtype=np.float32)
        s_w2 = np.zeros(T2 * P, dtype=np.float32)
        for g in range(NWIN):
            i = c * NWIN + g
            s0, s1 = starts2[i], starts2[i + 1]
            n = s1 - s0
            base = int(tile0[g]) * P
            q2[base:base + n] = q_s[s0:s1]
            s_dstloc2[base:base + n] = dslot2_s[s0:s1]
            s_w2[base:base + n] = w2_s[s0:s1]

        # xT: col wl*64 + s = x[node at (win c*WPC+wl, slot s)]
        per_core.append({
            "msgs": msgs_pt,
            "dstloc": to_pt(s_dstloc, np.float32, T1),
            "wts": to_pt(s_w, np.float32, T1),
            "qidx": _wrap_idx(q2),
            "dstloc2": to_pt(s_dstloc2, np.float32, T2),
            "wts2": to_pt(s_w2, np.float32, T2),
        })

    colmap = np.full((NCORES, WROWS), -1, dtype=np.int64)
    colmap[score, srow] = np.arange(N)
    for c in range(NCORES):
        cm = colmap[c]
        xT = np.zeros((D, WROWS), dtype=BF16)
        used = cm >= 0
        xT[:, used] = x[cm[used]].T.astype(BF16)
        per_core[c]["xT"] = xT
    return per_core, K1, K2, win, slot


def _l2_chunks(K2):
    """Split the 824 global window pairs into gather chunks of whole
    PSUM groups (L2_GROUP_PAIRS pairs) with <= L2_TILE_CAP tiles."""
    npairs = NWIN // 2
    pair_tiles = K2.reshape(npairs, 2).sum(axis=1)
    chunks = []  # (pair0, npair, tile0, ntiles) with groups inside
    p0 = 0
    while p0 < npairs:
        p1 = p0
        tiles = 0
        while p1 < npairs and p1 - p0 + L2_GROUP_PAIRS <= 48:
            # extend by whole groups of L2_GROUP_PAIRS pairs
            g1 = min(p1 + L2_GROUP_PAIRS, npairs)
            add = int(pair_tiles[p1:g1].sum())
            if tiles + add > L2_TILE_CAP and tiles > 0:
                break
            tiles += add
            p1 = g1
        chunks.append((p0, p1 - p0, tiles))
        p0 = p1
    return chunks


def _build_program(K1, K2, chunks):
    import concourse.bass as bass
    import concourse.tile as tile
    from concourse import bacc, mybir
    from concourse import library_config

    T1 = WPC * K1
    tile0 = np.zeros(NWIN + 1, dtype=np.int64)
    tile0[1:] = np.cumsum(K2)
    T2 = int(tile0[-1])

    nc = bacc.Bacc("TRN2", target_bir_lowering=False, debug=False,
                   num_devices=NCORES)
    dt = mybir.dt

    msgs_d = nc.dram_tensor("msgs", [P, T1 * D], dt.bfloat16, kind="ExternalInput")
    dstloc_d = nc.dram_tensor("dstloc", [P, T1], dt.float32, kind="ExternalInput")
    wts_d = nc.dram_tensor("wts", [P, T1], dt.float32, kind="ExternalInput")
    qidx_d = nc.dram_tensor("qidx", [P, T2 * 8], dt.int16, kind="ExternalInput")
    dstloc2_d = nc.dram_tensor("dstloc2", [P, T2], dt.float32, kind="ExternalInput")
    wts2_d = nc.dram_tensor("wts2", [P, T2], dt.float32, kind="ExternalInput")
    xT_d = nc.dram_tensor("xT", [D, WROWS], dt.bfloat16, kind="ExternalInput")
    iota_d = nc.dram_tensor("iota", [P, WIN], dt.bfloat16, kind="ExternalInput")
    ident_d = nc.dram_tensor("ident", [P, P], dt.bfloat16, kind="ExternalInput")
    w1l_d = nc.dram_tensor("w1lT", [D, D], dt.bfloat16, kind="ExternalInput")
    w1r_d = nc.dram_tensor("w1rT", [D, D], dt.bfloat16, kind="ExternalInput")
    w2l_d = nc.dram_tensor("w2lT", [D, D], dt.bfloat16, kind="ExternalInput")
    w2r_d = nc.dram_tensor("w2rT", [D, D], dt.bfloat16, kind="ExternalInput")
    b1_d = nc.dram_tensor("b1c", [D, 1], dt.float32, kind="ExternalInput")
    b2_d = nc.dram_tensor("b2c", [D, 1], dt.float32, kind="ExternalInput")
    out_d = nc.dram_tensor("out", [D, WROWS], dt.float32, kind="ExternalOutput")

    # L1 chunks of CHUNK1_W windows
    l1_chunks = []
    w0 = 0
    while w0 < WPC:
        cw = min(CHUNK1_W, WPC - w0)
        l1_chunks.append((w0, cw))
        w0 += cw

    with tile.TileContext(nc) as tc:
        with (
            tc.tile_pool(name="const", bufs=1) as cpool,
            tc.tile_pool(name="dram", bufs=1, space="DRAM") as dpool,
        ):
            iota_sb = cpool.tile([P, WIN], dt.bfloat16, tag="iota")
            ident_sb = cpool.tile([P, P], dt.bfloat16, tag="ident")
            w1l_sb = cpool.tile([D, D], dt.bfloat16, tag="w1l")
            w1r_sb = cpool.tile([D, D], dt.bfloat16, tag="w1r")
            w2l_sb = cpool.tile([D, D], dt.bfloat16, tag="w2l")
            w2r_sb = cpool.tile([D, D], dt.bfloat16, tag="w2r")
            b1_sb = cpool.tile([D, 1], dt.float32, tag="b1")
            b2_sb = cpool.tile([D, 1], dt.float32, tag="b2")
            h1T_sb = cpool.tile([D, WROWS], dt.bfloat16, tag="h1T")
            h1rows_sb = cpool.tile([P, PAIRS * D], dt.bfloat16, tag="h1rows")
            qidx_sb = cpool.tile([P, T2 * 8], dt.int16, tag="qidx")
            dstloc2_sb = cpool.tile([P, T2], dt.float32, tag="dstloc2")
            wts2_sb = cpool.tile([P, T2], dt.float32, tag="wts2")

            table_dram = dpool.tile([WROWS, P], dt.bfloat16, tag="table")
            partial_dram = dpool.tile([TBL_ROWS, D], dt.bfloat16, tag="partial")
            agg2_dram = dpool.tile([WROWS, D], dt.bfloat16, tag="agg2")

            # ---------------- layer 1 ----------------
            with (
                tc.tile_pool(name="l1c", bufs=1) as l1c,
                tc.tile_pool(name="ch", bufs=2) as chpool,
                tc.tile_pool(name="m1", bufs=16) as mpool,
                tc.tile_pool(name="sp1", bufs=3) as spool,
                tc.tile_pool(name="psA", bufs=2, space="PSUM") as psA,
                tc.tile_pool(name="psB", bufs=2, space="PSUM") as psB,
            ):
                dstloc_sb = l1c.tile([P, T1], dt.float32, tag="dstloc")
                wts_sb = l1c.tile([P, T1], dt.float32, tag="wts")
                xT_sb = l1c.tile([D, WROWS], dt.bfloat16, tag="xT")

                for t_sb, t_d in [(iota_sb, iota_d), (dstloc_sb, dstloc_d),
                                  (wts_sb, wts_d)]:
                    nc.sync.dma_start(out=t_sb[:], in_=t_d.ap())
                deferred = [(xT_sb, xT_d), (ident_sb, ident_d),
                            (w1l_sb, w1l_d), (w1r_sb, w1r_d), (b1_sb, b1_d),
                            (w2l_sb, w2l_d), (w2r_sb, w2r_d), (b2_sb, b2_d),
                            (qidx_sb, qidx_d), (dstloc2_sb, dstloc2_d),
                            (wts2_sb, wts2_d)]

                for w0, cw in l1_chunks:
                    ch = chpool.tile([P, CHUNK1_W * K1 * D], dt.bfloat16, tag="ch")
                    nc.sync.dma_start(
                        out=ch[:, :cw * K1 * D],
                        in_=msgs_d.ap()[:, w0 * K1 * D:(w0 + cw) * K1 * D])
                    if w0 == 0:
                        for t_sb, t_d in deferred:
                            nc.sync.dma_start(out=t_sb[:], in_=t_d.ap())
                    s0 = 0
                    while s0 < cw:
                        sw = min(SUPER1_W, cw - s0)
                        agg_ps = psA.tile([D, SUPER1_W * WIN], dt.float32, tag="agg")
                        for s in range(sw):
                            wi = w0 + s0 + s
                            for k in range(K1):
                                t = wi * K1 + k
                                mt = mpool.tile([P, WIN], dt.bfloat16, tag="M")
                                nc.vector.tensor_scalar(
                                    out=mt[:], in0=iota_sb[:],
                                    scalar1=dstloc_sb[:, t:t + 1],
                                    scalar2=wts_sb[:, t:t + 1],
                                    op0=mybir.AluOpType.is_equal,
                                    op1=mybir.AluOpType.mult)
                                woff = s0 + s
                                nc.tensor.matmul(
                                    out=agg_ps[:, s * WIN:(s + 1) * WIN],
                                    lhsT=ch[:, (woff * K1 + k) * D:
                                            (woff * K1 + k + 1) * D],
                                    rhs=mt[:], start=(k == 0), stop=(k == K1 - 1))
                        agg_sb = spool.tile([D, SUPER1_W * WIN], dt.bfloat16,
                                            tag="aggsb")
                        nc.scalar.copy(out=agg_sb[:, :sw * WIN],
                                       in_=agg_ps[:, :sw * WIN])
                        h_ps = psB.tile([D, SUPER1_W * WIN], dt.float32, tag="hps")
                        wabs = w0 + s0
                        nc.tensor.matmul(out=h_ps[:, :sw * WIN], lhsT=w1l_sb[:],
                                         rhs=agg_sb[:, :sw * WIN],
                                         start=True, stop=False)
                        nc.tensor.matmul(out=h_ps[:, :sw * WIN], lhsT=w1r_sb[:],
                                         rhs=xT_sb[:, wabs * WIN:(wabs + sw) * WIN],
                                         start=False, stop=True)
                        nc.scalar.activation(
                            out=h1T_sb[:, wabs * WIN:(wabs + sw) * WIN],
                            in_=h_ps[:, :sw * WIN],
                            func=mybir.ActivationFunctionType.Relu, bias=b1_sb[:])
                        s0 += sw

                # rows for the table: pair j -> local rows j*128 + p
                for j in range(PAIRS):
                    nc.sync.dma_start_transpose(
                        out=h1rows_sb[:, j * D:(j + 1) * D],
                        in_=h1T_sb[:, j * P:(j + 1) * P])
                nc.sync.dma_start(
                    out=table_dram[:].rearrange("(j p) e -> p j e", p=P)[:, :, 0:D],
                    in_=h1rows_sb[:].rearrange("p (j f) -> p j f", f=D))

            # ---------------- layer 2 (push + ReduceScatter) ----------------
            nc.gpsimd.load_library(library_config.mlp)
            with (
                tc.tile_pool(name="gq", bufs=2) as gqpool,
                tc.tile_pool(name="m2", bufs=16) as m2pool,
                tc.tile_pool(name="stg", bufs=2) as stgpool,
                tc.tile_pool(name="psP", bufs=3, space="PSUM") as psP,
            ):
                for p0, npair, ntiles in chunks:
                    t0 = int(tile0[2 * p0])
                    gq = gqpool.tile([P, L2_TILE_CAP * P], dt.bfloat16, tag="gq")
                    nc.gpsimd.dma_gather(
                        gq[:, :ntiles * P].rearrange("p (c e) -> p c e", e=P),
                        table_dram[:], qidx_sb[:, t0 * 8:(t0 + ntiles) * 8],
                        ntiles * P, ntiles * P, P, single_packet=False)
                    stg = stgpool.tile([P, 48 * D], dt.bfloat16, tag="stg")
                    g0 = 0
                    while g0 < npair:
                        gp = min(L2_GROUP_PAIRS, npair - g0)
                        pps = psP.tile([P, L2_GROUP_PAIRS * D], dt.float32,
                                       tag="pps")
                        for pr in range(gp):
                            pair = p0 + g0 + pr
                            for h in range(2):
                                g = 2 * pair + h
                                kk = int(K2[g])
                                tg = int(tile0[g])
                                for k in range(kk):
                                    t = tg + k
                                    mt = m2pool.tile([P, WIN], dt.bfloat16,
                                                     tag="M2")
                                    nc.vector.tensor_scalar(
                                        out=mt[:], in0=iota_sb[:],
                                        scalar1=dstloc2_sb[:, t:t + 1],
                                        scalar2=wts2_sb[:, t:t + 1],
                                        op0=mybir.AluOpType.is_equal,
                                        op1=mybir.AluOpType.mult)
                                    nc.tensor.matmul(
                                        out=pps[h * WIN:(h + 1) * WIN,
                                                pr * D:(pr + 1) * D],
                                        lhsT=mt[:],
                                        rhs=gq[:, (t - t0) * P:(t - t0) * P + D],
                                        start=(k == 0), stop=(k == kk - 1))
                        nc.scalar.copy(out=stg[:, g0 * D:(g0 + gp) * D],
                                       in_=pps[:, :gp * D])
                        g0 += gp
                    nc.sync.dma_start(
                        out=partial_dram[:].rearrange("(j p) f -> p j f", p=P)
                            [:, p0:p0 + npair, :],
                        in_=stg[:, :npair * D].rearrange("p (j f) -> p j f", f=D))

            nc.gpsimd.collective_compute(
                "ReduceScatter", mybir.AluOpType.add,
                replica_groups=[list(range(NCORES))],
                ins=[partial_dram[:]], outs=[agg2_dram[:]])

            # ---------------- final: transpose + W2 ----------------
            with (
                tc.tile_pool(name="fin", bufs=1) as fin,
                tc.tile_pool(name="ot", bufs=3) as opool,
                tc.tile_pool(name="psT", bufs=4, space="PSUM") as psT,
                tc.tile_pool(name="psC", bufs=2, space="PSUM") as psC,
            ):
                a2rows = fin.tile([P, PAIRS * D], dt.bfloat16, tag="a2rows")
                a2T = fin.tile([D, WROWS], dt.bfloat16, tag="a2T")
                nc.sync.dma_start(
                    out=a2rows[:].rearrange("p (j f) -> p j f", f=D),
                    in_=agg2_dram[:].rearrange("(j p) f -> p j f", p=P))
                for j in range(PAIRS):
                    tr = psT.tile([D, P], dt.bfloat16, tag="tr")
                    nc.tensor.transpose(out=tr[:], in_=a2rows[:, j * D:(j + 1) * D],
                                        identity=ident_sb[:])
                    nc.scalar.copy(out=a2T[:, j * P:(j + 1) * P], in_=tr[:])
                ncols = 2 * P
                for s0 in range(0, WROWS, ncols):
                    sw = min(ncols, WROWS - s0)
                    h_ps = psC.tile([D, ncols], dt.float32, tag="h2ps")
                    nc.tensor.matmul(out=h_ps[:, :sw], lhsT=w2l_sb[:],
                                     rhs=a2T[:, s0:s0 + sw], start=True, stop=False)
                    nc.tensor.matmul(out=h_ps[:, :sw], lhsT=w2r_sb[:],
                                     rhs=h1T_sb[:, s0:s0 + sw],
                                     start=False, stop=True)
                    ot = opool.tile([D, ncols], dt.float32, tag="ot")
                    nc.scalar.add(out=ot[:, :sw], in_=h_ps[:, :sw], add=b2_sb[:])
                    nc.sync.dma_start(out=out_d.ap()[:, s0:s0 + sw],
                                      in_=ot[:, :sw])

    nc.compile()
    return nc


def prepare(x, edge_index, W1l, W1r, b1, W2l, W2r, b2):
    per_core, K1, K2, win, slot = _host_prep(x, edge_index)
    iota = np.tile(np.arange(WIN, dtype=np.float32), (P, 1)).astype(BF16)
    ident = np.eye(P, dtype=np.float32).astype(BF16)
    common = {
        "iota": iota, "ident": ident,
        "w1lT": np.asarray(W1l, np.float32).T.astype(BF16).copy(),
        "w1rT": np.asarray(W1r, np.float32).T.astype(BF16).copy(),
        "w2lT": np.asarray(W2l, np.float32).T.astype(BF16).copy(),
        "w2rT": np.asarray(W2r, np.float32).T.astype(BF16).copy(),
        "b1c": np.asarray(b1, np.float32).reshape(D, 1).copy(),
        "b2c": np.asarray(b2, np.float32).reshape(D, 1).copy(),
    }
    in_maps = [{**common, **pc} for pc in per_core]
    chunks = _l2_chunks(K2)
    nc = _build_program(K1, K2, chunks)
    return nc, in_maps, win, slot


def kernel(x, edge_index, W1l, W1r, b1, W2l, W2r, b2):
    from concourse import bass_utils

    nc, in_maps, win, slot = prepare(x, edge_index, W1l, W1r, b1,
                                     W2l, W2r, b2)
    res = bass_utils.run_bass_kernel_spmd(nc, in_maps, list(range(NCORES)))

    out = np.empty((N, D), dtype=np.float32)
    cols = (win % WPC) * WIN + slot
    cores = win // WPC
    for c in range(NCORES):
        m = cores == c
        out[m] = res.results[c]["out"][:, cols[m]].T
    return out


# revision 16
# speedup vs baseline: 2.1508x; 1.1293x over previous
"""Trainium2 Bass kernel for 2-layer GraphSAGE (mean aggregation), v3.

8-core SPMD, 64-node windows (1648 global, 206/core), layer-1 pull with
host-pregathered messages, layer-2 push over local sources with a
ReduceScatter:

- L1: host slots each window's in-edges into K1 128-edge tiles and
  pre-gathers x[src] bf16; device builds [128,64] one-hot M tiles on DVE
  (is_equal x weight), PSUM-accumulates feature-major agg per 4-window
  super, W1 matmuls + bias + ReLU -> h1T [64, 13184] bf16.
- h1 rows: HWDGE dma-transposes per window pair -> h1rows [128, 103*64];
  one strided DMA writes a 256B-padded local row table [13184, 128] bf16.
- L2: edges grouped by SOURCE core; per global dst window, gather local
  h1 rows by int16 row index (batched dma_gather, 256B elems), build
  [128,64] one-hot M2 (dst slot x 1/cnt) and matmul with M2 as lhsT ->
  ROW-major partials [64r, 64f]; two windows pack one PSUM [128,*] tile;
  staged bf16 partials DMA to a [105472, 64] table; one ReduceScatter
  leaves each core its own rows' agg2.
- Final: load agg2 rows, PE-transpose per pair to feature-major, W2
  matmuls + bias -> out [64, 13184] fp32; host un-permutes.
"""
import sys

sys.path.insert(0, '/opt/trn_rl_repo')
import heapq

import numpy as np
import ml_dtypes

BF16 = ml_dtypes.bfloat16
N = 100000
D = 64
NCORES = 8
P = 128
WIN = 64                    # nodes per window
WPC = 206                   # windows per core
PAIRS = WPC // 2            # 103 window pairs per core
WROWS = WPC * WIN           # 13184 local rows
NWIN = NCORES * WPC         # 1648 global windows
TBL_ROWS = NCORES * WROWS   # 105472 rows in the partial table

CHUNK1_W = 12               # L1 windows per streamed msgs chunk
SUPER1_W = 4                # L1 windows per PSUM super (<= 4; 4*64=256 cols)
L2_TILE_CAP = 112           # max edge-tiles per L2 gather chunk
L2_GROUP_PAIRS = 4          # window pairs per L2 PSUM tile ([128, 4*64])
RS_SPLIT = 48               # local pairs in the first ReduceScatter chunk


def _balance_nodes(deg):
    """Assign nodes to NWIN windows of <=WIN nodes, balancing degree sums."""
    order = np.argsort(-deg, kind='stable')
    win = np.empty(N, dtype=np.int32)
    slot = np.empty(N, dtype=np.int32)
    counts = np.zeros(NWIN, dtype=np.int32)
    heap = [(0, w) for w in range(NWIN)]
    heapq.heapify(heap)
    for n in order:
        while True:
            load, w = heapq.heappop(heap)
            if counts[w] < WIN:
                break
        win[n] = w
        slot[n] = counts[w]
        counts[w] += 1
        load += int(deg[n])
        if counts[w] < WIN:
            heapq.heappush(heap, (load, w))
    return win, slot


def _wrap_idx(flat):
    """int16 gather index layout: per 128-idx tile, idx j -> partition j%16
    (replicated x8 across 128 partitions), column j//16."""
    nt = flat.shape[0] // P
    v = flat.reshape(nt, 8, 16).transpose(2, 0, 1).reshape(16, nt * 8)
    return np.tile(v, (8, 1)).astype(np.int16)


def _host_prep(x, edge_index):
    x = np.asarray(x, dtype=np.float32)
    edge_index = np.asarray(edge_index)
    src = edge_index[0].astype(np.int64)
    dst = edge_index[1].astype(np.int64)
    cnt = np.bincount(dst, minlength=N).astype(np.float32)
    inv = (1.0 / np.maximum(cnt, 1.0)).astype(np.float32)
    win, slot = _balance_nodes(cnt.astype(np.int64))
    srow = (win % WPC) * WIN + slot            # local table row of each node
    score = win // WPC                          # owning core of each node

    # ---- L1 (pull): per-core edge tiles grouped by dst window ----
    dwin = win[dst]
    wcnt = np.bincount(dwin, minlength=NWIN)
    K1 = max(1, int(np.max((wcnt + P - 1) // P)))
    T1 = WPC * K1

    order1 = np.lexsort((srow[src], dwin))
    dwin_s = dwin[order1]
    dslot_s = slot[dst][order1].astype(np.float32)
    w_s = inv[dst][order1]
    esrc_s = src[order1]
    starts1 = np.searchsorted(dwin_s, np.arange(NWIN + 1))

    # ---- L2 (push): per-core edges grouped by global dst window ----
    ecore = score[src]                          # core owning the edge source
    key2 = ecore.astype(np.int64) * NWIN + dwin
    order2 = np.lexsort((srow[src], key2))
    k2_s = key2[order2]
    q_s = srow[src][order2].astype(np.int16)    # local src row (gather idx)
    dslot2_s = slot[dst][order2].astype(np.float32)
    w2_s = inv[dst][order2]
    starts2 = np.searchsorted(k2_s, np.arange(NCORES * NWIN + 1))
    cnt2 = (starts2[1:] - starts2[:-1]).reshape(NCORES, NWIN)
    K2 = np.maximum(1, (cnt2.max(axis=0) + P - 1) // P).astype(np.int32)  # [NWIN]
    tile0 = np.zeros(NWIN + 1, dtype=np.int64)
    tile0[1:] = np.cumsum(K2)
    T2 = int(tile0[-1])

    per_core = []
    for c in range(NCORES):
        # L1 slots
        s_dstloc = np.full(T1 * P, -1.0, dtype=np.float32)
        s_w = np.zeros(T1 * P, dtype=np.float32)
        s_esrc = np.zeros(T1 * P, dtype=np.int64)
        for wl in range(WPC):
            g = c * WPC + wl
            s0, s1 = starts1[g], starts1[g + 1]
            n = s1 - s0
            base = wl * K1 * P
            s_dstloc[base:base + n] = dslot_s[s0:s1]
            s_w[base:base + n] = w_s[s0:s1]
            s_esrc[base:base + n] = esrc_s[s0:s1]

        def to_pt(a, dt, T):
            return np.ascontiguousarray(a.reshape(T, P).T.astype(dt))

        msgs = x[s_esrc].astype(BF16)
        msgs_pt = np.ascontiguousarray(
            msgs.reshape(T1, P, D).transpose(1, 0, 2).reshape(P, T1 * D))

        # L2 slots
        q2 = np.zeros(T2 * P, dtype=np.int16)
        s_dstloc2 = np.full(T2 * P, -1.0, dtype=np.float32)
        s_w2 = np.zeros(T2 * P, dtype=np.float32)
        for g in range(NWIN):
            i = c * NWIN + g
            s0, s1 = starts2[i], starts2[i + 1]
            n = s1 - s0
            base = int(tile0[g]) * P
            q2[base:base + n] = q_s[s0:s1]
            s_dstloc2[base:base + n] = dslot2_s[s0:s1]
            s_w2[base:base + n] = w2_s[s0:s1]

        # xT: col wl*64 + s = x[node at (win c*WPC+wl, slot s)]
        per_core.append({
            "msgs": msgs_pt,
            "dstloc": to_pt(s_dstloc, np.float32, T1),
            "wts": to_pt(s_w, np.float32, T1),
            "qidx": _wrap_idx(q2),
            "dstloc2": to_pt(s_dstloc2, np.float32, T2),
            "wts2": to_pt(s_w2, np.float32, T2),
        })

    colmap = np.full((NCORES, WROWS), -1, dtype=np.int64)
    colmap[score, srow] = np.arange(N)
    for c in range(NCORES):
        cm = colmap[c]
        xT = np.zeros((D, WROWS), dtype=BF16)
        used = cm >= 0
        xT[:, used] = x[cm[used]].T.astype(BF16)
        per_core[c]["xT"] = xT
    return per_core, K1, K2, win, slot


def _l2_chunks(K2):
    """Split the 824 global window pairs into gather chunks of whole
    PSUM groups (L2_GROUP_PAIRS pairs) with <= L2_TILE_CAP tiles.

    Two phases: first every core-range's local pairs [0, RS_SPLIT) (feeds
    the early ReduceScatter), then the rest."""
    pair_tiles = K2.reshape(NWIN // 2, 2).sum(axis=1)
    ranges = [(PAIRS * c, PAIRS * c + RS_SPLIT) for c in range(NCORES)]
    ranges += [(PAIRS * c + RS_SPLIT, PAIRS * (c + 1)) for c in range(NCORES)]
    chunks = []  # (pair0, npair, ntiles)
    for r0, r1 in ranges:
        p0 = r0
        while p0 < r1:
            p1 = p0
            tiles = 0
            while p1 < r1 and p1 - p0 + L2_GROUP_PAIRS <= 56:
                g1 = min(p1 + L2_GROUP_PAIRS, r1)
                add = int(pair_tiles[p1:g1].sum())
                if tiles + add > L2_TILE_CAP and tiles > 0:
                    break
                tiles += add
                p1 = g1
            chunks.append((p0, p1 - p0, tiles))
            p0 = p1
    return chunks


def _build_program(K1, K2, chunks):
    import concourse.bass as bass
    import concourse.tile as tile
    from concourse import bacc, mybir
    from concourse import library_config

    T1 = WPC * K1
    tile0 = np.zeros(NWIN + 1, dtype=np.int64)
    tile0[1:] = np.cumsum(K2)
    T2 = int(tile0[-1])

    nc = bacc.Bacc("TRN2", target_bir_lowering=False, debug=False,
                   num_devices=NCORES)
    dt = mybir.dt

    msgs_d = nc.dram_tensor("msgs", [P, T1 * D], dt.bfloat16, kind="ExternalInput")
    dstloc_d = nc.dram_tensor("dstloc", [P, T1], dt.float32, kind="ExternalInput")
    wts_d = nc.dram_tensor("wts", [P, T1], dt.float32, kind="ExternalInput")
    qidx_d = nc.dram_tensor("qidx", [P, T2 * 8], dt.int16, kind="ExternalInput")
    dstloc2_d = nc.dram_tensor("dstloc2", [P, T2], dt.float32, kind="ExternalInput")
    wts2_d = nc.dram_tensor("wts2", [P, T2], dt.float32, kind="ExternalInput")
    xT_d = nc.dram_tensor("xT", [D, WROWS], dt.bfloat16, kind="ExternalInput")
    iota_d = nc.dram_tensor("iota", [P, WIN], dt.bfloat16, kind="ExternalInput")
    ident_d = nc.dram_tensor("ident", [P, P], dt.bfloat16, kind="ExternalInput")
    w1l_d = nc.dram_tensor("w1lT", [D, D], dt.bfloat16, kind="ExternalInput")
    w1r_d = nc.dram_tensor("w1rT", [D, D], dt.bfloat16, kind="ExternalInput")
    w2l_d = nc.dram_tensor("w2lT", [D, D], dt.bfloat16, kind="ExternalInput")
    w2r_d = nc.dram_tensor("w2rT", [D, D], dt.bfloat16, kind="ExternalInput")
    b1_d = nc.dram_tensor("b1c", [D, 1], dt.float32, kind="ExternalInput")
    b2_d = nc.dram_tensor("b2c", [D, 1], dt.float32, kind="ExternalInput")
    out_d = nc.dram_tensor("out", [D, WROWS], dt.float32, kind="ExternalOutput")

    # L1 chunks of CHUNK1_W windows
    l1_chunks = []
    w0 = 0
    while w0 < WPC:
        cw = min(CHUNK1_W, WPC - w0)
        l1_chunks.append((w0, cw))
        w0 += cw

    with tile.TileContext(nc) as tc:
        with (
            tc.tile_pool(name="const", bufs=1) as cpool,
            tc.tile_pool(name="dram", bufs=1, space="DRAM") as dpool,
        ):
            iota_sb = cpool.tile([P, WIN], dt.bfloat16, tag="iota")
            ident_sb = cpool.tile([P, P], dt.bfloat16, tag="ident")
            w1l_sb = cpool.tile([D, D], dt.bfloat16, tag="w1l")
            w1r_sb = cpool.tile([D, D], dt.bfloat16, tag="w1r")
            w2l_sb = cpool.tile([D, D], dt.bfloat16, tag="w2l")
            w2r_sb = cpool.tile([D, D], dt.bfloat16, tag="w2r")
            b1_sb = cpool.tile([D, 1], dt.float32, tag="b1")
            b2_sb = cpool.tile([D, 1], dt.float32, tag="b2")
            h1T_sb = cpool.tile([D, WROWS], dt.bfloat16, tag="h1T")
            h1rows_sb = cpool.tile([P, PAIRS * D], dt.bfloat16, tag="h1rows")
            qidx_sb = cpool.tile([P, T2 * 8], dt.int16, tag="qidx")
            dstloc2_sb = cpool.tile([P, T2], dt.float32, tag="dstloc2")
            wts2_sb = cpool.tile([P, T2], dt.float32, tag="wts2")

            table_dram = dpool.tile([WROWS, P], dt.bfloat16, tag="table")
            # two partial tables, one per ReduceScatter chunk; rows are
            # core-major so the flat 8-way RS split lands on core boundaries
            NPA, NPB = RS_SPLIT, PAIRS - RS_SPLIT
            partialA = dpool.tile([NCORES * NPA * P, D], dt.bfloat16, tag="pA")
            partialB = dpool.tile([NCORES * NPB * P, D], dt.bfloat16, tag="pB")
            agg2A = dpool.tile([NPA * P, D], dt.bfloat16, tag="agg2A")
            agg2B = dpool.tile([NPB * P, D], dt.bfloat16, tag="agg2B")

            # ---------------- layer 1 ----------------
            with (
                tc.tile_pool(name="l1c", bufs=1) as l1c,
                tc.tile_pool(name="ch", bufs=2) as chpool,
                tc.tile_pool(name="m1", bufs=16) as mpool,
                tc.tile_pool(name="sp1", bufs=3) as spool,
                tc.tile_pool(name="psA", bufs=2, space="PSUM") as psA,
                tc.tile_pool(name="psB", bufs=2, space="PSUM") as psB,
            ):
                dstloc_sb = l1c.tile([P, T1], dt.float32, tag="dstloc")
                wts_sb = l1c.tile([P, T1], dt.float32, tag="wts")
                xT_sb = l1c.tile([D, WROWS], dt.bfloat16, tag="xT")

                for t_sb, t_d in [(iota_sb, iota_d), (dstloc_sb, dstloc_d),
                                  (wts_sb, wts_d)]:
                    nc.sync.dma_start(out=t_sb[:], in_=t_d.ap())
                # L2 consts load on idle engines during L1 (CoreSim charges
                # DMA transfers serially to the issuing engine)
                deferred_sp = [(ident_sb, ident_d), (w1l_sb, w1l_d),
                               (w1r_sb, w1r_d), (b1_sb, b1_d),
                               (w2l_sb, w2l_d), (w2r_sb, w2r_d), (b2_sb, b2_d)]
                deferred_pool = [(qidx_sb, qidx_d), (dstloc2_sb, dstloc2_d),
                                 (wts2_sb, wts2_d)]
                deferred_act = [(xT_sb, xT_d)]

                for w0, cw in l1_chunks:
                    ch = chpool.tile([P, CHUNK1_W * K1 * D], dt.bfloat16, tag="ch")
                    nc.sync.dma_start(
                        out=ch[:, :cw * K1 * D],
                        in_=msgs_d.ap()[:, w0 * K1 * D:(w0 + cw) * K1 * D])
                    if w0 == 0:
                        for t_sb, t_d in deferred_sp:
                            nc.sync.dma_start(out=t_sb[:], in_=t_d.ap())
                        for t_sb, t_d in deferred_pool:
                            nc.gpsimd.dma_start(out=t_sb[:], in_=t_d.ap())
                        for t_sb, t_d in deferred_act:
                            nc.scalar.dma_start(out=t_sb[:], in_=t_d.ap())
                    s0 = 0
                    while s0 < cw:
                        sw = min(SUPER1_W, cw - s0)
                        agg_ps = psA.tile([D, SUPER1_W * WIN], dt.float32, tag="agg")
                        for s in range(sw):
                            wi = w0 + s0 + s
                            for k in range(K1):
                                t = wi * K1 + k
                                mt = mpool.tile([P, WIN], dt.bfloat16, tag="M")
                                nc.vector.tensor_scalar(
                                    out=mt[:], in0=iota_sb[:],
                                    scalar1=dstloc_sb[:, t:t + 1],
                                    scalar2=wts_sb[:, t:t + 1],
                                    op0=mybir.AluOpType.is_equal,
                                    op1=mybir.AluOpType.mult)
                                woff = s0 + s
                                nc.tensor.matmul(
                                    out=agg_ps[:, s * WIN:(s + 1) * WIN],
                                    lhsT=ch[:, (woff * K1 + k) * D:
                                            (woff * K1 + k + 1) * D],
                                    rhs=mt[:], start=(k == 0), stop=(k == K1 - 1))
                        agg_sb = spool.tile([D, SUPER1_W * WIN], dt.bfloat16,
                                            tag="aggsb")
                        nc.scalar.copy(out=agg_sb[:, :sw * WIN],
                                       in_=agg_ps[:, :sw * WIN])
                        h_ps = psB.tile([D, SUPER1_W * WIN], dt.float32, tag="hps")
                        wabs = w0 + s0
                        nc.tensor.matmul(out=h_ps[:, :sw * WIN], lhsT=w1l_sb[:],
                                         rhs=agg_sb[:, :sw * WIN],
                                         start=True, stop=False)
                        nc.tensor.matmul(out=h_ps[:, :sw * WIN], lhsT=w1r_sb[:],
                                         rhs=xT_sb[:, wabs * WIN:(wabs + sw) * WIN],
                                         start=False, stop=True)
                        nc.scalar.activation(
                            out=h1T_sb[:, wabs * WIN:(wabs + sw) * WIN],
                            in_=h_ps[:, :sw * WIN],
                            func=mybir.ActivationFunctionType.Relu, bias=b1_sb[:])
                        s0 += sw

                # rows for the table: pair j -> local rows j*128 + p
                for j in range(PAIRS):
                    nc.sync.dma_start_transpose(
                        out=h1rows_sb[:, j * D:(j + 1) * D],
                        in_=h1T_sb[:, j * P:(j + 1) * P])
                nc.sync.dma_start(
                    out=table_dram[:].rearrange("(j p) e -> p j e", p=P)[:, :, 0:D],
                    in_=h1rows_sb[:].rearrange("p (j f) -> p j f", f=D))

            # ---------------- layer 2 (push + ReduceScatter) ----------------
            nc.gpsimd.load_library(library_config.mlp)
            stg_engines = [nc.sync, nc.scalar]
            with (
                tc.tile_pool(name="gq", bufs=2) as gqpool,
                tc.tile_pool(name="m2", bufs=16) as m2pool,
                tc.tile_pool(name="stg", bufs=2) as stgpool,
                tc.tile_pool(name="psP", bufs=3, space="PSUM") as psP,
            ):
                for ci, (p0, npair, ntiles) in enumerate(chunks):
                    t0 = int(tile0[2 * p0])
                    gq = gqpool.tile([P, L2_TILE_CAP * P], dt.bfloat16, tag="gq")
                    nc.gpsimd.dma_gather(
                        gq[:, :ntiles * P].rearrange("p (c e) -> p c e", e=P),
                        table_dram[:], qidx_sb[:, t0 * 8:(t0 + ntiles) * 8],
                        ntiles * P, ntiles * P, P, single_packet=False)
                    stg = stgpool.tile([P, 56 * D], dt.bfloat16, tag="stg")
                    g0 = 0
                    while g0 < npair:
                        gp = min(L2_GROUP_PAIRS, npair - g0)
                        pps = psP.tile([P, L2_GROUP_PAIRS * D], dt.float32,
                                       tag="pps")
                        for pr in range(gp):
                            pair = p0 + g0 + pr
                            for h in range(2):
                                g = 2 * pair + h
                                kk = int(K2[g])
                                tg = int(tile0[g])
                                for k in range(kk):
                                    t = tg + k
                                    mt = m2pool.tile([P, WIN], dt.bfloat16,
                                                     tag="M2")
                                    nc.vector.tensor_scalar(
                                        out=mt[:], in0=iota_sb[:],
                                        scalar1=dstloc2_sb[:, t:t + 1],
                                        scalar2=wts2_sb[:, t:t + 1],
                                        op0=mybir.AluOpType.is_equal,
                                        op1=mybir.AluOpType.mult)
                                    nc.tensor.matmul(
                                        out=pps[h * WIN:(h + 1) * WIN,
                                                pr * D:(pr + 1) * D],
                                        lhsT=mt[:],
                                        rhs=gq[:, (t - t0) * P:(t - t0) * P + D],
                                        start=(k == 0), stop=(k == kk - 1))
                        nc.scalar.copy(out=stg[:, g0 * D:(g0 + gp) * D],
                                       in_=pps[:, :gp * D])
                        g0 += gp
                    c, jl = p0 // PAIRS, p0 % PAIRS
                    if jl < RS_SPLIT:
                        tgt, row0 = partialA, c * NPA + jl
                    else:
                        tgt, row0 = partialB, c * NPB + (jl - RS_SPLIT)
                    stg_engines[ci % 2].dma_start(
                        out=tgt[:].rearrange("(j p) f -> p j f", p=P)
                            [:, row0:row0 + npair, :],
                        in_=stg[:, :npair * D].rearrange("p (j f) -> p j f", f=D))

            # -------- final: W2r*h1 during the collectives, then W2l --------
            SPLIT = RS_SPLIT * P
            with (
                tc.tile_pool(name="fin", bufs=1) as fin,
                tc.tile_pool(name="psT", bufs=2, space="PSUM") as psT,
                tc.tile_pool(name="psC", bufs=2, space="PSUM") as psC,
            ):
                a2rows = fin.tile([P, PAIRS * D], dt.bfloat16, tag="a2rows")
                a2T = fin.tile([D, WROWS], dt.bfloat16, tag="a2T")
                h2r = fin.tile([D, WROWS], dt.bfloat16, tag="h2r")
                ot = fin.tile([D, WROWS], dt.float32, tag="ot")

                # h2r = W2r @ h1 + b2 — independent of the collectives
                for s0 in range(0, WROWS, 512):
                    sw = min(512, WROWS - s0)
                    h_ps = psC.tile([D, 512], dt.float32, tag="h2rps")
                    nc.tensor.matmul(out=h_ps[:, :sw], lhsT=w2r_sb[:],
                                     rhs=h1T_sb[:, s0:s0 + sw],
                                     start=True, stop=True)
                    nc.scalar.add(out=h2r[:, s0:s0 + sw], in_=h_ps[:, :sw],
                                  add=b2_sb[:])

                nc.gpsimd.collective_compute(
                    "ReduceScatter", mybir.AluOpType.add,
                    replica_groups=[list(range(NCORES))],
                    ins=[partialA[:]], outs=[agg2A[:]])
                nc.gpsimd.collective_compute(
                    "ReduceScatter", mybir.AluOpType.add,
                    replica_groups=[list(range(NCORES))],
                    ins=[partialB[:]], outs=[agg2B[:]])

                halves = [(0, RS_SPLIT, agg2A), (RS_SPLIT, PAIRS, agg2B)]
                for j0, j1, a2d in halves:
                    nc.gpsimd.dma_start(
                        out=a2rows[:, j0 * D:j1 * D].rearrange(
                            "p (j f) -> p j f", f=D),
                        in_=a2d[:].rearrange("(j p) f -> p j f", p=P))
                    for jg in range(j0, j1, 4):
                        je = min(jg + 4, j1)
                        tr = psT.tile([D, 4 * P], dt.bfloat16, tag="tr")
                        for j in range(jg, je):
                            nc.tensor.transpose(
                                out=tr[:, (j - jg) * P:(j - jg + 1) * P],
                                in_=a2rows[:, j * D:(j + 1) * D],
                                identity=ident_sb[:])
                        nc.scalar.copy(out=a2T[:, jg * P:je * P],
                                       in_=tr[:, :(je - jg) * P])
                    for s0 in range(j0 * P, j1 * P, 512):
                        sw = min(512, j1 * P - s0)
                        h_ps = psC.tile([D, 512], dt.float32, tag="h2ps")
                        nc.tensor.matmul(out=h_ps[:, :sw], lhsT=w2l_sb[:],
                                         rhs=a2T[:, s0:s0 + sw],
                                         start=True, stop=True)
                        nc.vector.scalar_tensor_tensor(
                            out=ot[:, s0:s0 + sw], in0=h_ps[:, :sw],
                            scalar=1.0, in1=h2r[:, s0:s0 + sw],
                            op0=mybir.AluOpType.mult,
                            op1=mybir.AluOpType.add)
                    nc.scalar.dma_start(out=out_d.ap()[:, j0 * P:j1 * P],
                                        in_=ot[:, j0 * P:j1 * P])

    nc.compile()
    return nc


def prepare(x, edge_index, W1l, W1r, b1, W2l, W2r, b2):
    per_core, K1, K2, win, slot = _host_prep(x, edge_index)
    iota = np.tile(np.arange(WIN, dtype=np.float32), (P, 1)).astype(BF16)
    ident = np.eye(P, dtype=np.float32).astype(BF16)
    common = {
        "iota": iota, "ident": ident,
        "w1lT": np.asarray(W1l, np.float32).T.astype(BF16).copy(),
        "w1rT": np.asarray(W1r, np.float32).T.astype(BF16).copy(),
        "w2lT": np.asarray(W2l, np.float32).T.astype(BF16).copy(),
        "w2rT": np.asarray(W2r, np.float32).T.astype(BF16).copy(),
        "b1c": np.asarray(b1, np.float32).reshape(D, 1).copy(),
        "b2c": np.asarray(b2, np.float32).reshape(D, 1).copy(),
    }
    in_maps = [{**common, **pc} for pc in per_core]
    chunks = _l2_chunks(K2)
    nc = _build_program(K1, K2, chunks)
    return nc, in_maps, win, slot


def kernel(x, edge_index, W1l, W1r, b1, W2l, W2r, b2):
    from concourse import bass_utils

    nc, in_maps, win, slot = prepare(x, edge_index, W1l, W1r, b1,
                                     W2l, W2r, b2)
    res = bass_utils.run_bass_kernel_spmd(nc, in_maps, list(range(NCORES)))

    out = np.empty((N, D), dtype=np.float32)
    cols = (win % WPC) * WIN + slot
    cores = win // WPC
    for c in range(NCORES):
        m = cores == c
        out[m] = res.results[c]["out"][:, cols[m]].T
    return out


# revision 23
# speedup vs baseline: 2.3936x; 1.1129x over previous
"""Trainium2 Bass kernel for 2-layer GraphSAGE (mean aggregation), v3.

8-core SPMD, 64-node windows (1648 global, 206/core), layer-1 pull with
host-pregathered messages, layer-2 push over local sources with a
ReduceScatter:

- L1: host slots each window's in-edges into K1 128-edge tiles and
  pre-gathers x[src] bf16; device builds [128,64] one-hot M tiles on DVE
  (is_equal x weight), PSUM-accumulates feature-major agg per 4-window
  super, W1 matmuls + bias + ReLU -> h1T [64, 13184] bf16.
- h1 rows: HWDGE dma-transposes per window pair -> h1rows [128, 103*64];
  one strided DMA writes a 256B-padded local row table [13184, 128] bf16.
- L2: edges grouped by SOURCE core; per global dst window, gather local
  h1 rows by int16 row index (batched dma_gather, 256B elems), build
  [128,64] one-hot M2 (dst slot x 1/cnt) and matmul with M2 as lhsT ->
  ROW-major partials [64r, 64f]; two windows pack one PSUM [128,*] tile;
  staged bf16 partials DMA to a [105472, 64] table; one ReduceScatter
  leaves each core its own rows' agg2.
- Final: load agg2 rows, PE-transpose per pair to feature-major, W2
  matmuls + bias -> out [64, 13184] fp32; host un-permutes.
"""
import sys

sys.path.insert(0, '/opt/trn_rl_repo')
import heapq

import numpy as np
import ml_dtypes

BF16 = ml_dtypes.bfloat16
N = 100000
D = 64
NCORES = 8
P = 128
WIN = 64                    # nodes per window
WPC = 206                   # windows per core
PAIRS = WPC // 2            # 103 window pairs per core
WROWS = WPC * WIN           # 13184 local rows
NWIN = NCORES * WPC         # 1648 global windows
TBL_ROWS = NCORES * WROWS   # 105472 rows in the partial table

CHUNK1_W = 12               # L1 windows per streamed msgs chunk
SUPER1_W = 4                # L1 windows per PSUM super (<= 4; 4*64=256 cols)
L2_TILE_CAP = 112           # max edge-tiles per L2 gather chunk
L2_GROUP_PAIRS = 4          # window pairs per L2 PSUM tile ([128, 4*64])
RS_SPLIT = 48               # local pairs in the first ReduceScatter chunk


def _balance_nodes(deg):
    """Assign nodes to NWIN windows of <=WIN nodes, balancing degree sums."""
    order = np.argsort(-deg, kind='stable')
    win = np.empty(N, dtype=np.int32)
    slot = np.empty(N, dtype=np.int32)
    counts = np.zeros(NWIN, dtype=np.int32)
    heap = [(0, w) for w in range(NWIN)]
    heapq.heapify(heap)
    for n in order:
        while True:
            load, w = heapq.heappop(heap)
            if counts[w] < WIN:
                break
        win[n] = w
        slot[n] = counts[w]
        counts[w] += 1
        load += int(deg[n])
        if counts[w] < WIN:
            heapq.heappush(heap, (load, w))
    return win, slot


def _wrap_idx(flat):
    """int16 gather index layout: per 128-idx tile, idx j -> partition j%16
    (replicated x8 across 128 partitions), column j//16."""
    nt = flat.shape[0] // P
    v = flat.reshape(nt, 8, 16).transpose(2, 0, 1).reshape(16, nt * 8)
    return np.tile(v, (8, 1)).astype(np.int16)


def _host_prep(x, edge_index):
    x = np.asarray(x, dtype=np.float32)
    edge_index = np.asarray(edge_index)
    src = edge_index[0].astype(np.int64)
    dst = edge_index[1].astype(np.int64)
    cnt = np.bincount(dst, minlength=N).astype(np.float32)
    inv = (1.0 / np.maximum(cnt, 1.0)).astype(np.float32)
    win, slot = _balance_nodes(cnt.astype(np.int64))
    srow = (win % WPC) * WIN + slot            # local table row of each node
    score = win // WPC                          # owning core of each node

    # ---- L1 (pull): per-core edge tiles grouped by dst window ----
    dwin = win[dst]
    wcnt = np.bincount(dwin, minlength=NWIN)
    K1 = max(1, int(np.max((wcnt + P - 1) // P)))
    T1 = WPC * K1

    order1 = np.lexsort((srow[src], dwin))
    dwin_s = dwin[order1]
    dslot_s = slot[dst][order1].astype(np.float32)
    w_s = inv[dst][order1]
    esrc_s = src[order1]
    starts1 = np.searchsorted(dwin_s, np.arange(NWIN + 1))

    # ---- L2 (push): per-core edges grouped by global dst window ----
    ecore = score[src]                          # core owning the edge source
    key2 = ecore.astype(np.int64) * NWIN + dwin
    order2 = np.lexsort((srow[src], key2))
    k2_s = key2[order2]
    q_s = srow[src][order2].astype(np.int16)    # local src row (gather idx)
    dslot2_s = slot[dst][order2].astype(np.float32)
    w2_s = inv[dst][order2]
    starts2 = np.searchsorted(k2_s, np.arange(NCORES * NWIN + 1))
    cnt2 = (starts2[1:] - starts2[:-1]).reshape(NCORES, NWIN)
    K2 = np.maximum(1, (cnt2.max(axis=0) + P - 1) // P).astype(np.int32)  # [NWIN]
    tile0 = np.zeros(NWIN + 1, dtype=np.int64)
    tile0[1:] = np.cumsum(K2)
    T2 = int(tile0[-1])

    per_core = []
    for c in range(NCORES):
        # L1 slots
        s_dstloc = np.full(T1 * P, -1.0, dtype=np.float32)
        s_w = np.zeros(T1 * P, dtype=np.float32)
        s_esrc = np.zeros(T1 * P, dtype=np.int64)
        for wl in range(WPC):
            g = c * WPC + wl
            s0, s1 = starts1[g], starts1[g + 1]
            n = s1 - s0
            base = wl * K1 * P
            s_dstloc[base:base + n] = dslot_s[s0:s1]
            s_w[base:base + n] = w_s[s0:s1]
            s_esrc[base:base + n] = esrc_s[s0:s1]

        def to_pt(a, dt, T):
            return np.ascontiguousarray(a.reshape(T, P).T.astype(dt))

        msgs = x[s_esrc].astype(BF16)
        msgs_pt = np.ascontiguousarray(
            msgs.reshape(T1, P, D).transpose(1, 0, 2).reshape(P, T1 * D))

        # L2 slots
        q2 = np.zeros(T2 * P, dtype=np.int16)
        s_dstloc2 = np.full(T2 * P, -1.0, dtype=np.float32)
        s_w2 = np.zeros(T2 * P, dtype=np.float32)
        for g in range(NWIN):
            i = c * NWIN + g
            s0, s1 = starts2[i], starts2[i + 1]
            n = s1 - s0
            base = int(tile0[g]) * P
            q2[base:base + n] = q_s[s0:s1]
            s_dstloc2[base:base + n] = dslot2_s[s0:s1]
            s_w2[base:base + n] = w2_s[s0:s1]

        # local_scatter M1-build inputs: idx = k*WIN + dstloc (pad -> -1)
        dst_pt = to_pt(s_dstloc, np.float32, T1)
        w_pt = to_pt(s_w, np.float32, T1)
        k_of_t = np.tile(np.arange(K1, dtype=np.float32), WPC)
        ls_idx = np.where(dst_pt >= 0, dst_pt + k_of_t[None, :] * WIN,
                          -1.0).astype(np.int16)

        # xT: col wl*64 + s = x[node at (win c*WPC+wl, slot s)]
        per_core.append({
            "msgs": msgs_pt,
            "dstloc": dst_pt,
            "wts": w_pt,
            "lsidx": ls_idx,
            "lsw": w_pt.astype(BF16),
            "qidx": _wrap_idx(q2),
            "dstloc2": to_pt(s_dstloc2, np.float32, T2),
            "wts2": to_pt(s_w2, np.float32, T2),
        })

    colmap = np.full((NCORES, WROWS), -1, dtype=np.int64)
    colmap[score, srow] = np.arange(N)
    for c in range(NCORES):
        cm = colmap[c]
        xT = np.zeros((D, WROWS), dtype=BF16)
        used = cm >= 0
        xT[:, used] = x[cm[used]].T.astype(BF16)
        per_core[c]["xT"] = xT
    return per_core, K1, K2, win, slot


def _l2_chunks(K2):
    """Split the 824 global window pairs into gather chunks of whole
    PSUM groups (L2_GROUP_PAIRS pairs) with <= L2_TILE_CAP tiles.

    Two phases: first every core-range's local pairs [0, RS_SPLIT) (feeds
    the early ReduceScatter), then the rest."""
    pair_tiles = K2.reshape(NWIN // 2, 2).sum(axis=1)
    ranges = [(PAIRS * c, PAIRS * c + RS_SPLIT) for c in range(NCORES)]
    ranges += [(PAIRS * c + RS_SPLIT, PAIRS * (c + 1)) for c in range(NCORES)]
    chunks = []  # (pair0, npair, ntiles)
    for r0, r1 in ranges:
        p0 = r0
        while p0 < r1:
            p1 = p0
            tiles = 0
            while p1 < r1 and p1 - p0 + L2_GROUP_PAIRS <= 56:
                g1 = min(p1 + L2_GROUP_PAIRS, r1)
                add = int(pair_tiles[p1:g1].sum())
                if tiles + add > L2_TILE_CAP and tiles > 0:
                    break
                tiles += add
                p1 = g1
            chunks.append((p0, p1 - p0, tiles))
            p0 = p1
    return chunks


def _build_program(K1, K2, chunks):
    import concourse.bass as bass
    import concourse.tile as tile
    from concourse import bacc, mybir
    from concourse import library_config

    T1 = WPC * K1
    tile0 = np.zeros(NWIN + 1, dtype=np.int64)
    tile0[1:] = np.cumsum(K2)
    T2 = int(tile0[-1])

    nc = bacc.Bacc("TRN2", target_bir_lowering=False, debug=False,
                   num_devices=NCORES)
    dt = mybir.dt

    msgs_d = nc.dram_tensor("msgs", [P, T1 * D], dt.bfloat16, kind="ExternalInput")
    dstloc_d = nc.dram_tensor("dstloc", [P, T1], dt.float32, kind="ExternalInput")
    wts_d = nc.dram_tensor("wts", [P, T1], dt.float32, kind="ExternalInput")
    lsidx_d = nc.dram_tensor("lsidx", [P, T1], dt.int16, kind="ExternalInput")
    lsw_d = nc.dram_tensor("lsw", [P, T1], dt.bfloat16, kind="ExternalInput")
    qidx_d = nc.dram_tensor("qidx", [P, T2 * 8], dt.int16, kind="ExternalInput")
    dstloc2_d = nc.dram_tensor("dstloc2", [P, T2], dt.float32, kind="ExternalInput")
    wts2_d = nc.dram_tensor("wts2", [P, T2], dt.float32, kind="ExternalInput")
    xT_d = nc.dram_tensor("xT", [D, WROWS], dt.bfloat16, kind="ExternalInput")
    iota_d = nc.dram_tensor("iota", [P, WIN], dt.bfloat16, kind="ExternalInput")
    ident_d = nc.dram_tensor("ident", [P, P], dt.bfloat16, kind="ExternalInput")
    w1l_d = nc.dram_tensor("w1lT", [D, D], dt.bfloat16, kind="ExternalInput")
    w1r_d = nc.dram_tensor("w1rT", [D, D], dt.bfloat16, kind="ExternalInput")
    w2l_d = nc.dram_tensor("w2lT", [D, D], dt.bfloat16, kind="ExternalInput")
    w2r_d = nc.dram_tensor("w2rT", [D, D], dt.bfloat16, kind="ExternalInput")
    b1_d = nc.dram_tensor("b1c", [D, 1], dt.float32, kind="ExternalInput")
    b2_d = nc.dram_tensor("b2c", [D, 1], dt.float32, kind="ExternalInput")
    out_d = nc.dram_tensor("out", [D, WROWS], dt.float32, kind="ExternalOutput")

    # L1 chunks of CHUNK1_W windows
    l1_chunks = []
    w0 = 0
    while w0 < WPC:
        cw = min(CHUNK1_W, WPC - w0)
        l1_chunks.append((w0, cw))
        w0 += cw

    with tile.TileContext(nc) as tc:
        with (
            tc.tile_pool(name="const", bufs=1) as cpool,
            tc.tile_pool(name="dram", bufs=1, space="DRAM") as dpool,
        ):
            iota_sb = cpool.tile([P, WIN], dt.bfloat16, tag="iota")
            ident_sb = cpool.tile([P, P], dt.bfloat16, tag="ident")
            w1l_sb = cpool.tile([D, D], dt.bfloat16, tag="w1l")
            w1r_sb = cpool.tile([D, D], dt.bfloat16, tag="w1r")
            w2l_sb = cpool.tile([D, D], dt.bfloat16, tag="w2l")
            w2r_sb = cpool.tile([D, D], dt.bfloat16, tag="w2r")
            b1_sb = cpool.tile([D, 1], dt.float32, tag="b1")
            b2_sb = cpool.tile([D, 1], dt.float32, tag="b2")
            h1T_sb = cpool.tile([D, WROWS], dt.bfloat16, tag="h1T")
            h1rows_sb = cpool.tile([P, PAIRS * D], dt.bfloat16, tag="h1rows")
            qidx_sb = cpool.tile([P, T2 * 8], dt.int16, tag="qidx")
            dstloc2_sb = cpool.tile([P, T2], dt.float32, tag="dstloc2")
            wts2_sb = cpool.tile([P, T2], dt.float32, tag="wts2")

            table_dram = dpool.tile([WROWS, P], dt.bfloat16, tag="table")
            # two partial tables, one per ReduceScatter chunk; rows are
            # core-major so the flat 8-way RS split lands on core boundaries
            NPA, NPB = RS_SPLIT, PAIRS - RS_SPLIT
            partialA = dpool.tile([NCORES * NPA * P, D], dt.bfloat16, tag="pA")
            partialB = dpool.tile([NCORES * NPB * P, D], dt.bfloat16, tag="pB")
            agg2A = dpool.tile([NPA * P, D], dt.bfloat16, tag="agg2A")
            agg2B = dpool.tile([NPB * P, D], dt.bfloat16, tag="agg2B")

            # ---------------- layer 1 ----------------
            with (
                tc.tile_pool(name="l1c", bufs=1) as l1c,
                tc.tile_pool(name="ch", bufs=2) as chpool,
                tc.tile_pool(name="m1", bufs=16) as mpool,
                tc.tile_pool(name="sp1", bufs=3) as spool,
                tc.tile_pool(name="psA", bufs=2, space="PSUM") as psA,
                tc.tile_pool(name="psB", bufs=2, space="PSUM") as psB,
            ):
                dstloc_sb = l1c.tile([P, T1], dt.float32, tag="dstloc")
                wts_sb = l1c.tile([P, T1], dt.float32, tag="wts")
                lsidx_sb = l1c.tile([P, T1], dt.int16, tag="lsidx")
                lsw_sb = l1c.tile([P, T1], dt.bfloat16, tag="lsw")
                xT_sb = l1c.tile([D, WROWS], dt.bfloat16, tag="xT")

                for t_sb, t_d in [(iota_sb, iota_d), (dstloc_sb, dstloc_d),
                                  (wts_sb, wts_d)]:
                    nc.sync.dma_start(out=t_sb[:], in_=t_d.ap())
                for t_sb, t_d in [(lsidx_sb, lsidx_d), (lsw_sb, lsw_d)]:
                    nc.scalar.dma_start(out=t_sb[:], in_=t_d.ap())
                # L2 consts load on idle engines during L1 (CoreSim charges
                # DMA transfers serially to the issuing engine)
                deferred_sp = [(ident_sb, ident_d), (w1l_sb, w1l_d),
                               (w1r_sb, w1r_d), (b1_sb, b1_d),
                               (w2l_sb, w2l_d), (w2r_sb, w2r_d), (b2_sb, b2_d)]
                deferred_pool = [(qidx_sb, qidx_d), (dstloc2_sb, dstloc2_d),
                                 (wts2_sb, wts2_d)]
                deferred_act = [(xT_sb, xT_d)]

                for w0, cw in l1_chunks:
                    ch = chpool.tile([P, CHUNK1_W * K1 * D], dt.bfloat16, tag="ch")
                    nc.sync.dma_start(
                        out=ch[:, :cw * K1 * D],
                        in_=msgs_d.ap()[:, w0 * K1 * D:(w0 + cw) * K1 * D])
                    if w0 == 0:
                        for t_sb, t_d in deferred_sp:
                            nc.sync.dma_start(out=t_sb[:], in_=t_d.ap())
                        for t_sb, t_d in deferred_pool:
                            nc.gpsimd.dma_start(out=t_sb[:], in_=t_d.ap())
                        for t_sb, t_d in deferred_act:
                            nc.scalar.dma_start(out=t_sb[:], in_=t_d.ap())
                    s0 = 0
                    while s0 < cw:
                        sw = min(SUPER1_W, cw - s0)
                        agg_ps = psA.tile([D, SUPER1_W * WIN], dt.float32, tag="agg")
                        for s in range(sw):
                            wi = w0 + s0 + s
                            if wi % 2 == 0:
                                # whole-window M build on the idle Pool engine
                                mwin = mpool.tile([P, K1 * WIN], dt.bfloat16,
                                                  tag="Mw")
                                nc.gpsimd.local_scatter(
                                    out_ap=mwin[:],
                                    data_ap=lsw_sb[:, wi * K1:(wi + 1) * K1],
                                    idxs_ap=lsidx_sb[:, wi * K1:(wi + 1) * K1],
                                    channels=P, num_elems=K1 * WIN,
                                    num_idxs=K1)
                            for k in range(K1):
                                t = wi * K1 + k
                                if wi % 2 == 0:
                                    mt = mwin[:, k * WIN:(k + 1) * WIN]
                                else:
                                    m1t = mpool.tile([P, WIN], dt.bfloat16,
                                                     tag="M")
                                    nc.vector.tensor_scalar(
                                        out=m1t[:], in0=iota_sb[:],
                                        scalar1=dstloc_sb[:, t:t + 1],
                                        scalar2=wts_sb[:, t:t + 1],
                                        op0=mybir.AluOpType.is_equal,
                                        op1=mybir.AluOpType.mult)
                                    mt = m1t[:]
                                woff = s0 + s
                                nc.tensor.matmul(
                                    out=agg_ps[:, s * WIN:(s + 1) * WIN],
                                    lhsT=ch[:, (woff * K1 + k) * D:
                                            (woff * K1 + k + 1) * D],
                                    rhs=mt, start=(k == 0), stop=(k == K1 - 1))
                        agg_sb = spool.tile([D, SUPER1_W * WIN], dt.bfloat16,
                                            tag="aggsb")
                        nc.scalar.copy(out=agg_sb[:, :sw * WIN],
                                       in_=agg_ps[:, :sw * WIN])
                        h_ps = psB.tile([D, SUPER1_W * WIN], dt.float32, tag="hps")
                        wabs = w0 + s0
                        nc.tensor.matmul(out=h_ps[:, :sw * WIN], lhsT=w1l_sb[:],
                                         rhs=agg_sb[:, :sw * WIN],
                                         start=True, stop=False)
                        nc.tensor.matmul(out=h_ps[:, :sw * WIN], lhsT=w1r_sb[:],
                                         rhs=xT_sb[:, wabs * WIN:(wabs + sw) * WIN],
                                         start=False, stop=True)
                        nc.scalar.activation(
                            out=h1T_sb[:, wabs * WIN:(wabs + sw) * WIN],
                            in_=h_ps[:, :sw * WIN],
                            func=mybir.ActivationFunctionType.Relu, bias=b1_sb[:])
                        s0 += sw

                # rows for the table: pair j -> local rows j*128 + p
                for j in range(PAIRS):
                    nc.sync.dma_start_transpose(
                        out=h1rows_sb[:, j * D:(j + 1) * D],
                        in_=h1T_sb[:, j * P:(j + 1) * P])
                nc.sync.dma_start(
                    out=table_dram[:].rearrange("(j p) e -> p j e", p=P)[:, :, 0:D],
                    in_=h1rows_sb[:].rearrange("p (j f) -> p j f", f=D))

            # ---------------- layer 2 (push + ReduceScatter) ----------------
            nc.gpsimd.load_library(library_config.mlp)
            stg_engines = [nc.sync, nc.scalar]
            with (
                tc.tile_pool(name="gq", bufs=2) as gqpool,
                tc.tile_pool(name="m2", bufs=16) as m2pool,
                tc.tile_pool(name="stg", bufs=2) as stgpool,
                tc.tile_pool(name="psP", bufs=3, space="PSUM") as psP,
            ):
                for ci, (p0, npair, ntiles) in enumerate(chunks):
                    t0 = int(tile0[2 * p0])
                    gq = gqpool.tile([P, L2_TILE_CAP * P], dt.bfloat16, tag="gq")
                    nc.gpsimd.dma_gather(
                        gq[:, :ntiles * P].rearrange("p (c e) -> p c e", e=P),
                        table_dram[:], qidx_sb[:, t0 * 8:(t0 + ntiles) * 8],
                        ntiles * P, ntiles * P, P, single_packet=False)
                    stg = stgpool.tile([P, 56 * D], dt.bfloat16, tag="stg")
                    g0 = 0
                    while g0 < npair:
                        gp = min(L2_GROUP_PAIRS, npair - g0)
                        pps = psP.tile([P, L2_GROUP_PAIRS * D], dt.float32,
                                       tag="pps")
                        for pr in range(gp):
                            pair = p0 + g0 + pr
                            for h in range(2):
                                g = 2 * pair + h
                                kk = int(K2[g])
                                tg = int(tile0[g])
                                for k in range(kk):
                                    t = tg + k
                                    mt = m2pool.tile([P, WIN], dt.bfloat16,
                                                     tag="M2")
                                    nc.vector.tensor_scalar(
                                        out=mt[:], in0=iota_sb[:],
                                        scalar1=dstloc2_sb[:, t:t + 1],
                                        scalar2=wts2_sb[:, t:t + 1],
                                        op0=mybir.AluOpType.is_equal,
                                        op1=mybir.AluOpType.mult)
                                    nc.tensor.matmul(
                                        out=pps[h * WIN:(h + 1) * WIN,
                                                pr * D:(pr + 1) * D],
                                        lhsT=mt[:],
                                        rhs=gq[:, (t - t0) * P:(t - t0) * P + D],
                                        start=(k == 0), stop=(k == kk - 1))
                        nc.scalar.copy(out=stg[:, g0 * D:(g0 + gp) * D],
                                       in_=pps[:, :gp * D])
                        g0 += gp
                    c, jl = p0 // PAIRS, p0 % PAIRS
                    if jl < RS_SPLIT:
                        tgt, row0 = partialA, c * NPA + jl
                    else:
                        tgt, row0 = partialB, c * NPB + (jl - RS_SPLIT)
                    stg_engines[ci % 2].dma_start(
                        out=tgt[:].rearrange("(j p) f -> p j f", p=P)
                            [:, row0:row0 + npair, :],
                        in_=stg[:, :npair * D].rearrange("p (j f) -> p j f", f=D))

            # -------- final: W2r*h1 during the collectives, then W2l --------
            SPLIT = RS_SPLIT * P
            with (
                tc.tile_pool(name="fin", bufs=1) as fin,
                tc.tile_pool(name="psT", bufs=2, space="PSUM") as psT,
                tc.tile_pool(name="psC", bufs=2, space="PSUM") as psC,
            ):
                a2rows = fin.tile([P, PAIRS * D], dt.bfloat16, tag="a2rows")
                a2T = fin.tile([D, WROWS], dt.bfloat16, tag="a2T")
                h2r = fin.tile([D, WROWS], dt.bfloat16, tag="h2r")
                ot = fin.tile([D, WROWS], dt.float32, tag="ot")

                # h2r = W2r @ h1 + b2 — independent of the collectives
                for s0 in range(0, WROWS, 512):
                    sw = min(512, WROWS - s0)
                    h_ps = psC.tile([D, 512], dt.float32, tag="h2rps")
                    nc.tensor.matmul(out=h_ps[:, :sw], lhsT=w2r_sb[:],
                                     rhs=h1T_sb[:, s0:s0 + sw],
                                     start=True, stop=True)
                    nc.scalar.add(out=h2r[:, s0:s0 + sw], in_=h_ps[:, :sw],
                                  add=b2_sb[:])

                nc.gpsimd.collective_compute(
                    "ReduceScatter", mybir.AluOpType.add,
                    replica_groups=[list(range(NCORES))],
                    ins=[partialA[:]], outs=[agg2A[:]])
                nc.gpsimd.collective_compute(
                    "ReduceScatter", mybir.AluOpType.add,
                    replica_groups=[list(range(NCORES))],
                    ins=[partialB[:]], outs=[agg2B[:]])

                halves = [(0, RS_SPLIT, agg2A), (RS_SPLIT, PAIRS, agg2B)]
                for j0, j1, a2d in halves:
                    nc.sync.dma_start(
                        out=a2rows[:, j0 * D:j1 * D].rearrange(
                            "p (j f) -> p j f", f=D),
                        in_=a2d[:].rearrange("(j p) f -> p j f", p=P))
                    for jg in range(j0, j1, 4):
                        je = min(jg + 4, j1)
                        tr = psT.tile([D, 4 * P], dt.bfloat16, tag="tr")
                        for j in range(jg, je):
                            nc.tensor.transpose(
                                out=tr[:, (j - jg) * P:(j - jg + 1) * P],
                                in_=a2rows[:, j * D:(j + 1) * D],
                                identity=ident_sb[:])
                        nc.scalar.copy(out=a2T[:, jg * P:je * P],
                                       in_=tr[:, :(je - jg) * P])
                    for s0 in range(j0 * P, j1 * P, 512):
                        sw = min(512, j1 * P - s0)
                        h_ps = psC.tile([D, 512], dt.float32, tag="h2ps")
                        nc.tensor.matmul(out=h_ps[:, :sw], lhsT=w2l_sb[:],
                                         rhs=a2T[:, s0:s0 + sw],
                                         start=True, stop=True)
                        nc.vector.scalar_tensor_tensor(
                            out=ot[:, s0:s0 + sw], in0=h_ps[:, :sw],
                            scalar=1.0, in1=h2r[:, s0:s0 + sw],
                            op0=mybir.AluOpType.mult,
                            op1=mybir.AluOpType.add)
                    nc.sync.dma_start(out=out_d.ap()[:, j0 * P:j1 * P],
                                      in_=ot[:, j0 * P:j1 * P])

    nc.compile()
    return nc


def prepare(x, edge_index, W1l, W1r, b1, W2l, W2r, b2):
    per_core, K1, K2, win, slot = _host_prep(x, edge_index)
    iota = np.tile(np.arange(WIN, dtype=np.float32), (P, 1)).astype(BF16)
    ident = np.eye(P, dtype=np.float32).astype(BF16)
    common = {
        "iota": iota, "ident": ident,
        "w1lT": np.asarray(W1l, np.float32).T.astype(BF16).copy(),
        "w1rT": np.asarray(W1r, np.float32).T.astype(BF16).copy(),
        "w2lT": np.asarray(W2l, np.float32).T.astype(BF16).copy(),
        "w2rT": np.asarray(W2r, np.float32).T.astype(BF16).copy(),
        "b1c": np.asarray(b1, np.float32).reshape(D, 1).copy(),
        "b2c": np.asarray(b2, np.float32).reshape(D, 1).copy(),
    }
    in_maps = [{**common, **pc} for pc in per_core]
    chunks = _l2_chunks(K2)
    nc = _build_program(K1, K2, chunks)
    return nc, in_maps, win, slot


def kernel(x, edge_index, W1l, W1r, b1, W2l, W2r, b2):
    from concourse import bass_utils

    nc, in_maps, win, slot = prepare(x, edge_index, W1l, W1r, b1,
                                     W2l, W2r, b2)
    res = bass_utils.run_bass_kernel_spmd(nc, in_maps, list(range(NCORES)))

    out = np.empty((N, D), dtype=np.float32)
    cols = (win % WPC) * WIN + slot
    cores = win // WPC
    for c in range(NCORES):
        m = cores == c
        out[m] = res.results[c]["out"][:, cols[m]].T
    return out
